# revision 14
# baseline (speedup 1.0000x reference)
"""BitLinear 1-bit (BitNet-style) linear layer on 8 Trainium2 NeuronCores.

y = x_q @ Wb^T where
  x_q = per-token group-64 absmax int8 fake-quant of x
  Wb  = per-row centered binarization: sign(W - rowmean) * rowmean(|W - rowmean|)

Sharding: data-parallel over tokens. Each core gets a 1024-token slice of x
(full 4096-feature rows) plus the full W, computes its y^T shard
[4096 out, 1024 tok] so the per-row alpha scale is a per-partition scalar,
and the host concatenates + transposes.

Key kernel choices:
  - matmul runs in bf16: the weight side is exactly +-1 (exact in bf16, alpha
    factored out of the matmul and applied at PSUM eviction); only x_q is
    rounded to bf16 (measured ~1.2e-3 absmax relative error vs f32 reference).
  - round() is the f32 magic-number trick (+1.5*2^23, -1.5*2^23 = RNE) as a
    dual-op tensor_scalar. clip is a no-op since |x/scale*127| <= 127 by
    construction.
  - x_q^T and S^T = sign(W-m)^T are produced by xbar DMA transposes issued
    from sync's otherwise-idle HWDGE; the PE stream is pure matmul.
  - DMA split: x loads + transposes on sync's queue, W loads + y stores on
    scalar's queue, so the x/transpose path is not stuck behind 67MB of W.
  - W-prep runs with lookahead so S^T inventory stays ahead of the in-order
    PE stream; the first o-block sweeps token-chunk-outer so chunk-1 matmuls
    don't block the PE while quant is still running.
"""

import sys

sys.path.insert(0, "/opt/trn_rl_repo")

import numpy as np

import concourse.bacc as bacc
import concourse.tile as tile
from concourse import mybir
from concourse.bass_utils import run_bass_kernel_spmd

F32 = mybir.dt.float32
BF16 = mybir.dt.bfloat16
AX = mybir.AxisListType
ALU = mybir.AluOpType
ACTF = mybir.ActivationFunctionType

MAGIC = 1.5 * 2**23  # adding+subtracting forces RNE round-to-integer in f32
QMAX = 127.0
EPS = 1e-8
GROUP = 64

N_CORES = 8
B, S, D_IN, D_OUT = 4, 2048, 4096, 4096
T_TOTAL = B * S


def build_program(T=1024, D=4096, O=4096, tchunk=512, pre=3, ahead=2):
    """Emit the per-core program. T tokens x [O, D] weight -> yT [O, T]."""
    P = 128
    nt = T // P          # token tiles
    nk = D // P          # contraction (k) blocks
    no = O // P          # output-row tiles
    ntc = T // tchunk    # token chunks per matmul sweep
    ng = D // GROUP      # quant groups per token row
    pre = min(pre, no)

    nc = bacc.Bacc(None, target_bir_lowering=False)

    x_d = nc.dram_tensor("x", [T, D], F32, kind="ExternalInput")
    w_d = nc.dram_tensor("W", [O, D], F32, kind="ExternalInput")
    y_d = nc.dram_tensor("yT", [O, T], F32, kind="ExternalOutput")

    with tile.TileContext(nc) as tc:
        with (
            tc.tile_pool(name="xin", bufs=2) as xinp,
            tc.tile_pool(name="bsc", bufs=3) as bscp,  # q + sign tiles, shared
            tc.tile_pool(name="sc", bufs=3) as scp,
            tc.tile_pool(name="xqt", bufs=1) as xqtp,
            tc.tile_pool(name="win", bufs=2) as winp,
            tc.tile_pool(name="st", bufs=pre + ahead + 1) as stp,
            tc.tile_pool(name="wsc", bufs=pre + ahead + 4) as wscp,
            tc.tile_pool(name="yout", bufs=2) as youtp,
            tc.tile_pool(name="yp", bufs=4, space="PSUM") as ypp,
        ):
            # x_q^T stays resident in SBUF: [128, nk, T] bf16 (d on partitions)
            xqt = xqtp.tile([P, nk, T], BF16)

            x_t = x_d.rearrange("(n p) d -> n p d", p=P)
            w_t = w_d.rearrange("(n p) d -> n p d", p=P)

            sts = {}
            alphas = {}

            def w_prep(o):
                """Binarize W rows [o*128, (o+1)*128): -> S^T tile + alpha."""
                wt = winp.tile([P, D], F32, tag="wt")
                nc.scalar.dma_start(wt[:], w_t[o])

                sg = bscp.tile([P, D], BF16, tag="bsc")
                msum = wscp.tile([P, 1], F32, tag="msum")
                nc.vector.tensor_reduce(msum[:], wt[:], axis=AX.X, op=ALU.add)
                negm = wscp.tile([P, 1], F32, tag="negm")
                nc.vector.tensor_scalar(
                    negm[:], msum[:], -1.0 / D, None, op0=ALU.mult
                )
                # alpha = mean(|W - m|) via ACT Abs with accum (sg is scratch)
                asum = wscp.tile([P, 1], F32, tag="asum")
                nc.scalar.activation(
                    sg[:], wt[:], ACTF.Abs, bias=negm[:, 0:1], accum_out=asum[:]
                )
                alpha = wscp.tile([P, 1], F32, tag="alpha")
                nc.vector.tensor_scalar(
                    alpha[:], asum[:], 1.0 / D, None, op0=ALU.mult
                )
                # S = sign(W - m) in {-1, +1}, exact in bf16
                nc.scalar.activation(sg[:], wt[:], ACTF.Sign, bias=negm[:, 0:1])

                # S^T via one xbar DMA: st[p, m, j] = S[j, m*128+p]
                st = stp.tile([P, nk, P], BF16, tag="st")
                nc.sync.dma_start_transpose(st[:], sg[:])
                sts[o] = st
                alphas[o] = alpha

            next_prep = 0

            def ensure_prep(upto):
                nonlocal next_prep
                while next_prep <= min(upto, no - 1):
                    w_prep(next_prep)
                    next_prep += 1

            def quant(t):
                """Fake-quantize token tile t and append x_q^T columns."""
                xt = xinp.tile([P, D], F32, tag="xt")
                nc.sync.dma_start(xt[:], x_t[t])
                xg = xt[:].rearrange("p (g e) -> p g e", e=GROUP)

                # group absmax -> scale; sr = max(scale,eps)/127 ; rs = 1/sr
                amax = scp.tile([P, ng], F32, tag="amax")
                nc.vector.tensor_reduce(
                    amax[:], xg, axis=AX.X, op=ALU.max, apply_absolute_value=True
                )
                sr = scp.tile([P, ng], F32, tag="sr")
                nc.vector.tensor_scalar(
                    sr[:], amax[:], EPS, 1.0 / QMAX, op0=ALU.max, op1=ALU.mult
                )
                rs = scp.tile([P, ng], F32, tag="rs")
                nc.vector.reciprocal(rs[:], sr[:])

                # t1 = x * rs (broadcast rs over the 64-wide group), in place
                rs_b = rs[:].unsqueeze(-1).broadcast_to((P, ng, GROUP))
                nc.vector.tensor_tensor(xg, xg, rs_b, op=ALU.mult)
                # q = RNE-round(t1); q in [-127,127] ints, exact in bf16
                q = bscp.tile([P, D], BF16, tag="bsc")
                nc.vector.tensor_scalar(
                    q[:], xt[:], MAGIC, MAGIC, op0=ALU.add, op1=ALU.subtract
                )
                # xq = q * sr -> bf16, in place over q
                qg = q[:].rearrange("p (g e) -> p g e", e=GROUP)
                sr_b = sr[:].unsqueeze(-1).broadcast_to((P, ng, GROUP))
                nc.vector.tensor_tensor(qg, qg, sr_b, op=ALU.mult)

                # transpose via one xbar DMA: dst[p, m, j] = q[j, m*128+p]
                nc.sync.dma_start_transpose(
                    xqt[:, :, t * P : (t + 1) * P], q[:]
                )

            def mm_sweep(o, t2):
                """yT[o-tile, chunk t2] = (S^T).T @ xq^T, k-accumulated."""
                yp = ypp.tile([P, tchunk], F32, tag="yp")
                st = sts[o]
                for k in range(nk):
                    nc.tensor.matmul(
                        yp[:],
                        st[:, k, :],
                        xqt[:, k, t2 * tchunk : (t2 + 1) * tchunk],
                        start=(k == 0),
                        stop=(k == nk - 1),
                    )
                yo = youtp.tile([P, tchunk], F32, tag="yo")
                # evict + fold in alpha (per-partition scale)
                nc.vector.tensor_scalar(
                    yo[:], yp[:], alphas[o][:, 0:1], None, op0=ALU.mult
                )
                nc.scalar.dma_start(
                    y_d[o * P : (o + 1) * P, t2 * tchunk : (t2 + 1) * tchunk],
                    yo[:],
                )

            # ---- startup: S^T(0) first, quant of the first token chunk,
            # more W-prep, rest of quant.
            ensure_prep(0)
            first_chunk_tiles = min(tchunk // P, nt)
            for t in range(first_chunk_tiles):
                quant(t)
            ensure_prep(pre - 1)
            for t in range(first_chunk_tiles, nt):
                quant(t)

            # ---- first o-block sweeps chunk-outer: all chunk-0 matmuls for
            # `pre` o-tiles run before any chunk-1 matmul blocks the PE
            # stream; later W-preps are interleaved to keep inventory ahead.
            for i, (t2, o) in enumerate(
                [(t2, o) for t2 in range(ntc) for o in range(pre)]
            ):
                ensure_prep(pre - 1 + i)
                mm_sweep(o, t2)

            # ---- steady state: keep `ahead` S^T tiles of lookahead.
            for o in range(pre, no):
                ensure_prep(o + ahead)
                for t2 in range(ntc):
                    mm_sweep(o, t2)

    nc.compile()
    return nc


_cached = {}

# set by test harnesses: when True, capture an NTFF trace of core 0 and stash
# the BassKernelResults (with exec_time_ns) in LAST_RESULTS.
TRACE = False
LAST_RESULTS = None


def _get_program(key):
    if key not in _cached:
        T, D, O, tchunk = key
        _cached[key] = build_program(T, D, O, tchunk)
    return _cached[key]


def kernel(x: np.ndarray, W: np.ndarray) -> np.ndarray:
    assert x.shape == (B, S, D_IN) and W.shape == (D_OUT, D_IN)
    x2 = np.ascontiguousarray(x.reshape(T_TOTAL, D_IN), dtype=np.float32)
    Wc = np.ascontiguousarray(W, dtype=np.float32)

    t_core = T_TOTAL // N_CORES
    nc = _get_program((t_core, D_IN, D_OUT, 512))

    in_maps = [
        {"x": x2[i * t_core : (i + 1) * t_core], "W": Wc} for i in range(N_CORES)
    ]
    global LAST_RESULTS
    res = run_bass_kernel_spmd(
        nc, in_maps, core_ids=list(range(N_CORES)), trace=TRACE
    )
    LAST_RESULTS = res

    y2 = np.empty((T_TOTAL, D_OUT), dtype=np.float32)
    for i in range(N_CORES):
        y2[i * t_core : (i + 1) * t_core, :] = res.results[i]["yT"].T
    return y2.reshape(B, S, D_OUT)


# revision 15
# speedup vs baseline: 1.1971x; 1.1971x over previous
"""BitLinear 1-bit (BitNet-style) linear layer on 8 Trainium2 NeuronCores.

y = x_q @ Wb^T where
  x_q = per-token group-64 absmax int8 fake-quant of x
  Wb  = per-row centered binarization: sign(W - rowmean) * rowmean(|W - rowmean|)

Sharding: data-parallel over tokens. Each core gets a 1024-token slice of x
(full 4096-feature rows) plus the full W, computes its y^T shard
[4096 out, 1024 tok] so the per-row alpha scale is a per-partition scalar,
and the host concatenates + transposes.

Key kernel choices:
  - matmul runs in bf16: the weight side is exactly +-1 (exact in bf16, alpha
    factored out of the matmul and applied at PSUM eviction); only x_q is
    rounded to bf16 (measured ~1.2e-3 absmax relative error vs f32 reference).
  - round() is the f32 magic-number trick (+1.5*2^23, -1.5*2^23 = RNE) as a
    dual-op tensor_scalar. clip is a no-op since |x/scale*127| <= 127 by
    construction.
  - x_q^T and S^T = sign(W-m)^T are produced by PE transposes (bf16, ~55ns
    warm per 128x128 block, hidden among matmuls). Measured faster than xbar
    DMA transposes, whose descriptor generation serializes an HWDGE queue.
  - W-prep runs with lookahead so S^T inventory stays ahead of the in-order
    PE stream; the first o-block sweeps token-chunk-outer so chunk-1 matmuls
    don't block the PE while quant is still running.
"""

import sys

sys.path.insert(0, "/opt/trn_rl_repo")

import numpy as np

import concourse.bacc as bacc
import concourse.tile as tile
from concourse import mybir
from concourse.bass_utils import run_bass_kernel_spmd
from concourse.masks import make_identity

F32 = mybir.dt.float32
BF16 = mybir.dt.bfloat16
AX = mybir.AxisListType
ALU = mybir.AluOpType
ACTF = mybir.ActivationFunctionType

MAGIC = 1.5 * 2**23  # adding+subtracting forces RNE round-to-integer in f32
QMAX = 127.0
EPS = 1e-8
GROUP = 64

N_CORES = 8
B, S, D_IN, D_OUT = 4, 2048, 4096, 4096
T_TOTAL = B * S


def build_program(T=1024, D=4096, O=4096, tchunk=512, pre=4, ahead=2):
    """Emit the per-core program. T tokens x [O, D] weight -> yT [O, T]."""
    P = 128
    nt = T // P          # token tiles
    nk = D // P          # contraction (k) blocks
    no = O // P          # output-row tiles
    ntc = T // tchunk    # token chunks per matmul sweep
    ng = D // GROUP      # quant groups per token row
    kb = 4               # k-blocks per PSUM bank (4 * [128,128] bf16)
    pre = min(pre, no)

    nc = bacc.Bacc(None, target_bir_lowering=False)

    x_d = nc.dram_tensor("x", [T, D], F32, kind="ExternalInput")
    w_d = nc.dram_tensor("W", [O, D], F32, kind="ExternalInput")
    y_d = nc.dram_tensor("yT", [O, T], F32, kind="ExternalOutput")

    with tile.TileContext(nc) as tc:
        with (
            tc.tile_pool(name="const", bufs=1) as constp,
            tc.tile_pool(name="xin", bufs=2) as xinp,
            tc.tile_pool(name="bsc", bufs=3) as bscp,  # q + sign tiles, shared
            tc.tile_pool(name="sc", bufs=3) as scp,
            tc.tile_pool(name="xqt", bufs=1) as xqtp,
            tc.tile_pool(name="win", bufs=2) as winp,
            tc.tile_pool(name="st", bufs=pre + ahead) as stp,
            tc.tile_pool(name="wsc", bufs=pre + ahead + 4) as wscp,
            tc.tile_pool(name="yout", bufs=2) as youtp,
            tc.tile_pool(name="tp", bufs=4, space="PSUM") as tpp,
            tc.tile_pool(name="yp", bufs=2, space="PSUM") as ypp,
        ):
            ident = constp.tile([P, P], BF16)
            make_identity(nc, ident[:])

            # x_q^T stays resident in SBUF: [128, nk, T] bf16 (d on partitions)
            xqt = xqtp.tile([P, nk, T], BF16)

            x_t = x_d.rearrange("(n p) d -> n p d", p=P)
            w_t = w_d.rearrange("(n p) d -> n p d", p=P)

            sts = {}
            alphas = {}

            def pe_transpose(src, dst3, evict):
                """dst3[p, k, :] = src[:, k*128:(k+1)*128].T for all nk blocks.

                PE transposes into PSUM (4 blocks per bank), then `evict`
                engine copies to SBUF.
                """
                for m in range(nk // kb):
                    tp = tpp.tile([P, kb * P], BF16, tag="tp")
                    for j in range(kb):
                        k = m * kb + j
                        nc.tensor.transpose(
                            tp[:, j * P : (j + 1) * P],
                            src[:, k * P : (k + 1) * P],
                            ident[:],
                        )
                    dst = dst3[:, m * kb : (m + 1) * kb, :]
                    srcv = tp[:].rearrange("p (j c) -> p j c", c=P)
                    if evict == "act":
                        nc.scalar.copy(dst, srcv)
                    else:
                        nc.vector.tensor_copy(dst, srcv)

            def w_prep(o):
                """Binarize W rows [o*128, (o+1)*128): -> S^T tile + alpha."""
                wt = winp.tile([P, D], F32, tag="wt")
                nc.sync.dma_start(wt[:], w_t[o])

                sg = bscp.tile([P, D], BF16, tag="bsc")
                msum = wscp.tile([P, 1], F32, tag="msum")
                nc.vector.tensor_reduce(msum[:], wt[:], axis=AX.X, op=ALU.add)
                negm = wscp.tile([P, 1], F32, tag="negm")
                nc.vector.tensor_scalar(
                    negm[:], msum[:], -1.0 / D, None, op0=ALU.mult
                )
                # alpha = mean(|W - m|) via ACT Abs with accum (sg is scratch)
                asum = wscp.tile([P, 1], F32, tag="asum")
                nc.scalar.activation(
                    sg[:], wt[:], ACTF.Abs, bias=negm[:, 0:1], accum_out=asum[:]
                )
                alpha = wscp.tile([P, 1], F32, tag="alpha")
                nc.vector.tensor_scalar(
                    alpha[:], asum[:], 1.0 / D, None, op0=ALU.mult
                )
                # S = sign(W - m) in {-1, +1}, exact in bf16
                nc.scalar.activation(sg[:], wt[:], ACTF.Sign, bias=negm[:, 0:1])

                st = stp.tile([P, nk, P], BF16, tag="st")
                pe_transpose(sg[:], st, evict="dve")
                sts[o] = st
                alphas[o] = alpha

            next_prep = 0

            def ensure_prep(upto):
                nonlocal next_prep
                while next_prep <= min(upto, no - 1):
                    w_prep(next_prep)
                    next_prep += 1

            def quant(t):
                """Fake-quantize token tile t and append x_q^T columns."""
                xt = xinp.tile([P, D], F32, tag="xt")
                nc.sync.dma_start(xt[:], x_t[t])
                xg = xt[:].rearrange("p (g e) -> p g e", e=GROUP)

                # group absmax -> scale; sr = max(scale,eps)/127 ; rs = 1/sr
                amax = scp.tile([P, ng], F32, tag="amax")
                nc.vector.tensor_reduce(
                    amax[:], xg, axis=AX.X, op=ALU.max, apply_absolute_value=True
                )
                sr = scp.tile([P, ng], F32, tag="sr")
                nc.vector.tensor_scalar(
                    sr[:], amax[:], EPS, 1.0 / QMAX, op0=ALU.max, op1=ALU.mult
                )
                rs = scp.tile([P, ng], F32, tag="rs")
                nc.vector.reciprocal(rs[:], sr[:])

                # t1 = x * rs (broadcast rs over the 64-wide group), in place
                rs_b = rs[:].unsqueeze(-1).broadcast_to((P, ng, GROUP))
                nc.vector.tensor_tensor(xg, xg, rs_b, op=ALU.mult)
                # q = RNE-round(t1); q in [-127,127] ints, exact in bf16
                q = bscp.tile([P, D], BF16, tag="bsc")
                nc.vector.tensor_scalar(
                    q[:], xt[:], MAGIC, MAGIC, op0=ALU.add, op1=ALU.subtract
                )
                # xq = q * sr -> bf16, in place over q
                qg = q[:].rearrange("p (g e) -> p g e", e=GROUP)
                sr_b = sr[:].unsqueeze(-1).broadcast_to((P, ng, GROUP))
                nc.vector.tensor_tensor(qg, qg, sr_b, op=ALU.mult)

                dst = xqt[:, :, t * P : (t + 1) * P]
                pe_transpose(q[:], dst, evict="act")

            def mm_sweep(o, t2):
                """yT[o-tile, chunk t2] = (S^T).T @ xq^T, k-accumulated."""
                yp = ypp.tile([P, tchunk], F32, tag="yp")
                st = sts[o]
                for k in range(nk):
                    nc.tensor.matmul(
                        yp[:],
                        st[:, k, :],
                        xqt[:, k, t2 * tchunk : (t2 + 1) * tchunk],
                        start=(k == 0),
                        stop=(k == nk - 1),
                    )
                yo = youtp.tile([P, tchunk], F32, tag="yo")
                # evict + fold in alpha (per-partition scale)
                nc.scalar.activation(yo[:], yp[:], ACTF.Copy, scale=alphas[o][:, 0:1])
                nc.sync.dma_start(
                    y_d[o * P : (o + 1) * P, t2 * tchunk : (t2 + 1) * tchunk],
                    yo[:],
                )

            # ---- startup: S^T(0) first, quant of the first token chunk,
            # more W-prep, rest of quant.
            ensure_prep(0)
            first_chunk_tiles = min(tchunk // P, nt)
            for t in range(first_chunk_tiles):
                quant(t)
            ensure_prep(pre - 1)
            for t in range(first_chunk_tiles, nt):
                quant(t)

            # ---- first o-block sweeps chunk-outer: all chunk-0 matmuls for
            # `pre` o-tiles run before any chunk-1 matmul blocks the PE
            # stream; later W-preps are interleaved to keep inventory ahead.
            for i, (t2, o) in enumerate(
                [(t2, o) for t2 in range(ntc) for o in range(pre)]
            ):
                ensure_prep(pre - 1 + (i + 1) // 2)
                mm_sweep(o, t2)

            # ---- steady state: keep `ahead` S^T tiles of lookahead.
            for o in range(pre, no):
                ensure_prep(o + ahead)
                for t2 in range(ntc):
                    mm_sweep(o, t2)

    nc.compile()
    return nc


_cached = {}

# set by test harnesses: when True, capture an NTFF trace of core 0 and stash
# the BassKernelResults (with exec_time_ns) in LAST_RESULTS.
TRACE = False
LAST_RESULTS = None


def _get_program(key):
    if key not in _cached:
        T, D, O, tchunk = key
        _cached[key] = build_program(T, D, O, tchunk)
    return _cached[key]


def kernel(x: np.ndarray, W: np.ndarray) -> np.ndarray:
    assert x.shape == (B, S, D_IN) and W.shape == (D_OUT, D_IN)
    x2 = np.ascontiguousarray(x.reshape(T_TOTAL, D_IN), dtype=np.float32)
    Wc = np.ascontiguousarray(W, dtype=np.float32)

    t_core = T_TOTAL // N_CORES
    nc = _get_program((t_core, D_IN, D_OUT, 512))

    in_maps = [
        {"x": x2[i * t_core : (i + 1) * t_core], "W": Wc} for i in range(N_CORES)
    ]
    global LAST_RESULTS
    res = run_bass_kernel_spmd(
        nc, in_maps, core_ids=list(range(N_CORES)), trace=TRACE
    )
    LAST_RESULTS = res

    y2 = np.empty((T_TOTAL, D_OUT), dtype=np.float32)
    for i in range(N_CORES):
        y2[i * t_core : (i + 1) * t_core, :] = res.results[i]["yT"].T
    return y2.reshape(B, S, D_OUT)
